# revision 24
# baseline (speedup 1.0000x reference)
"""MoNet (GMMConv GNN) distributed Trainium2 kernel, v2.

Strategy (8 NeuronCores):
  - Nodes partitioned into 8 contiguous blocks of B=6250 (core m owns dests
    [m*B,(m+1)*B)).  Edges bucketed by (dest block of 128, source half) and
    padded to 128-edge tiles, so each core's segment-sum over its dest block
    is fully local and source indices fit int16.
  - Per layer: each core computes its block of xg = h @ Wg (row-padded to 128
    cols, fp16), AllGather -> full fp16 xg table in DRAM, per-edge gather of
    source rows via gpsimd dma_gather (256B rows), gaussian-weighted
    segment-sum as one-hot matmuls accumulating in PSUM per dest block of 128
    nodes, fused with the root-weight matmul.  The aggregation matmul is laid
    out transposed (lhsT=xj, rhs=sel) so the accumulator is [NHID, dest] and
    the relu+bias epilogue needs no transpose.
  - Gaussian edge weights are precomputed per layer on the host (they depend
    only on input degrees and layer constants), shipped as fp16 alongside the
    dest-lane id, so no activation-function tables are needed on device.
  - Host does index prep only: degree/dinv, edge sorting/padding, per-core
    edge tables. All O(N*F) and O(E*F) math runs on device in fp16 with fp32
    PSUM accumulation.
"""

import sys
from contextlib import ExitStack

import numpy as np

if "/opt/trn_rl_repo" not in sys.path:
    sys.path.insert(0, "/opt/trn_rl_repo")

import concourse.bacc as bacc
import concourse.mybir as mybir
import concourse.tile as tile
from concourse import bass_utils
from concourse import library_config

F32 = mybir.dt.float32
F16 = mybir.dt.float16
I16 = mybir.dt.int16
AF = mybir.ActivationFunctionType
ALU = mybir.AluOpType

P = 128
EPS = 1e-15


class Cfg:
    def __init__(self, N=50000, E=800000, NFEAT=128, NHID=96, NCLASS=40, NL=2, C=8):
        self.N, self.E, self.NFEAT, self.NHID, self.NCLASS = N, E, NFEAT, NHID, NCLASS
        self.NL, self.C = NL, C
        assert N % C == 0
        self.B = N // C
        self.NBLK = (self.B + P - 1) // P
        self.HALF = N // 2
        # xg table row padded to a 256B-multiple row (fp16)
        self.XGW = ((NHID + 127) // 128) * 128
        self.MAXT = 7           # gather tiles per dma_gather call
        self.DMA_SCRATCH = 16384  # SWDGE ring: 1024 descriptors


def host_prep(cfg, edge_index, edge_weight, Wp, bp, mu, sigma):
    """Edges bucketed by (dest block, source half) for int16 dma_gather;
    per-edge gaussian weights precomputed per layer."""
    N, C, B, NBLK, HALF = cfg.N, cfg.C, cfg.B, cfg.NBLK, cfg.HALF
    row = np.asarray(edge_index[0]).astype(np.int64)
    col = np.asarray(edge_index[1]).astype(np.int64)
    ew = np.asarray(edge_weight).astype(np.float64)
    deg = np.bincount(row, weights=ew, minlength=N)
    with np.errstate(divide="ignore"):
        dinv = np.where(deg > 0, 1.0 / np.sqrt(deg), 0.0)

    half = (row >= HALF).astype(np.int64)
    core = col // B
    loc = col - core * B
    blk = loc // P
    order = np.lexsort((blk, half, core))
    rs, cs = row[order], col[order]
    hs = half[order]
    core, loc, blk = core[order], loc[order], blk[order]
    dl = (loc - blk * P).astype(np.float32)

    # per-layer gaussian edge weights (float64 on host, stored fp16)
    Wp = np.asarray(Wp, np.float64)
    bp = np.asarray(bp, np.float64)
    mu = np.asarray(mu, np.float64)
    sigma = np.asarray(sigma, np.float64)
    u, v = dinv[rs], dinv[cs]
    gauss = []
    for i in range(cfg.NL):
        p = np.tanh(u * Wp[i, 0, 0] + v * Wp[i, 1, 0] + bp[i, 0])
        d = p - mu[i, 0, 0]
        gauss.append(np.exp(-0.5 * d * d / (EPS + sigma[i, 0, 0] ** 2)))

    NG = NBLK * 2
    g = hs * NBLK + blk  # group within core, half-major (merged gather runs)
    cnt = np.zeros((C, NG), np.int64)
    np.add.at(cnt, (core, g), 1)
    K = ((cnt + P - 1) // P).max(axis=0)  # [NG] tiles per (blk, half)
    toff = np.concatenate([[0], np.cumsum(K)]).astype(np.int64)
    T = int(toff[-1])

    gg = core * NG + g
    gcnt = np.bincount(gg, minlength=C * NG)
    gstart = np.concatenate([[0], np.cumsum(gcnt)])[:-1]
    idx_in_g = np.arange(len(gg)) - gstart[gg]
    lane = (idx_in_g % P).astype(np.int64)
    tloc = idx_in_g // P               # tile within the (blk, half) call
    tcol = (toff[g] + tloc).astype(np.int64)

    # ed layout: [gauss_l0 | gauss_l1 | dl], fp16.  dl sentinel -1 on padded
    # lanes kills them in the is_equal select.
    edA = np.zeros((C, P, 3 * T), np.float16)
    edA[:, :, 2 * T:3 * T] = -1.0
    edA[core, lane, tcol] = gauss[0].astype(np.float16)
    edA[core, lane, T + tcol] = gauss[1].astype(np.float16)
    edA[core, lane, 2 * T + tcol] = dl.astype(np.float16)
    # int16 idx in wrapped-16 layout: flat k = tloc*128 + lane within a call;
    # element k at [k % 16, call_off*8 + k // 16]; pad = row 0 (valid index;
    # sel kills it via dl=-1).  Shipped compact [16, 8T]; replicated to 128
    # partitions on device.
    idxA = np.zeros((C, 16, 8 * T), np.int16)
    k = tloc * P + lane
    r16 = (k % 16).astype(np.int64)
    c16 = (toff[g] * 8 + k // 16).astype(np.int64)
    idxA[core, r16, c16] = (rs - hs * HALF).astype(np.int16)
    return dict(idxA=idxA, edA=edA, K=[int(x) for x in K],
                toff=[int(x) for x in toff], T=T)


def build(cfg, prep):
    NHID, NCLASS, NFEAT = cfg.NHID, cfg.NCLASS, cfg.NFEAT
    B, NBLK, NL, C, XGW = cfg.B, cfg.NBLK, cfg.NL, cfg.C, cfg.XGW
    T = prep["T"]
    HALF = cfg.HALF
    K2, toff = prep["K"], prep["toff"]
    Kmax = max(max(K2), 1)
    MAXT = cfg.MAXT
    assert MAXT * P <= cfg.DMA_SCRATCH // 16

    nc = bacc.Bacc("TRN2", target_bir_lowering=False, debug=False, num_devices=C,
                   dynamic_dma_scratch_size=cfg.DMA_SCRATCH)
    hT_in = nc.declare_dram_parameter("hT", [NFEAT, B], F16, isOutput=False)
    idx_in = nc.declare_dram_parameter("idx16", [16, 8 * T], I16, isOutput=False)
    ed_in = nc.declare_dram_parameter("ed", [P, 3 * T], F16, isOutput=False)
    R_in = nc.declare_dram_parameter("R", [P, P], F32, isOutput=False)
    Wemb_in = nc.declare_dram_parameter("Wemb", [NFEAT, NHID], F16, isOutput=False)
    Wg_in = nc.declare_dram_parameter("Wg", [NL, NHID, XGW], F16, isOutput=False)
    Wr_in = nc.declare_dram_parameter("Wr", [NL, NHID, NHID], F16, isOutput=False)
    Wo_in = nc.declare_dram_parameter("Wo", [NHID, NCLASS], F16, isOutput=False)
    bemb_in = nc.declare_dram_parameter("bemb", [NHID, 1], F32, isOutput=False)
    bconv_in = nc.declare_dram_parameter("bconv", [NHID, NL], F32, isOutput=False)
    bout_in = nc.declare_dram_parameter("bout", [P, NCLASS], F32, isOutput=False)
    out_ext = nc.declare_dram_parameter("out", [B, NCLASS], F32, isOutput=True)

    with tile.TileContext(nc) as tc, ExitStack() as ctx:
        nc.gpsimd.load_library(library_config.mlp)
        const = ctx.enter_context(tc.tile_pool(name="const", bufs=1))
        sbp = ctx.enter_context(tc.tile_pool(name="sbp", bufs=3))
        xjp = ctx.enter_context(tc.tile_pool(name="xjp", bufs=4))
        selp = ctx.enter_context(tc.tile_pool(name="selp", bufs=16))
        hp = ctx.enter_context(tc.tile_pool(name="hp", bufs=2))
        pag = ctx.enter_context(tc.tile_pool(name="pag", bufs=4, space="PSUM"))
        pmm = ctx.enter_context(tc.tile_pool(name="pmm", bufs=3, space="PSUM"))
        dramp = ctx.enter_context(tc.tile_pool(name="dramp", bufs=1, space="DRAM"))

        def cload(ap, shape, dtype=F16, name=None):
            t = const.tile(shape, dtype, name=name or "c")
            nc.sync.dma_start(out=t[:], in_=ap)
            return t

        hT_s = cload(hT_in[:, :], [NFEAT, B], name="hT_s")
        ed_s = cload(ed_in[:, :], [P, 3 * T], name="ed_s")
        ed32 = const.tile([P, 3 * T], F32, name="ed32")
        nc.vector.tensor_copy(out=ed32[:], in_=ed_s[:])
        dl_s = ed32[:, 2 * T:3 * T]
        gl_s = [ed32[:, 0:T], ed32[:, T:2 * T]]
        R_s = cload(R_in[:, :], [P, P], F32, name="R_s")
        Wemb_s = cload(Wemb_in[:, :], [NFEAT, NHID], name="Wemb_s")
        Wo_s = cload(Wo_in[:, :], [NHID, NCLASS], name="Wo_s")
        bemb_s = cload(bemb_in[:, :], [NHID, 1], F32, name="bemb_s")
        bconv_s = cload(bconv_in[:, :], [NHID, NL], F32, name="bconv_s")
        bout_s = cload(bout_in[:, :], [P, NCLASS], F32, name="bout_s")
        Wg_s = const.tile([NHID, NL * XGW], F16, name="Wg_s")
        Wr_s = const.tile([NHID, NL * NHID], F16, name="Wr_s")
        for i in range(NL):
            nc.sync.dma_start(out=Wg_s[:, i * XGW:(i + 1) * XGW], in_=Wg_in[i])
            nc.sync.dma_start(out=Wr_s[:, i * NHID:(i + 1) * NHID], in_=Wr_in[i])
        # replicate compact idx rows [16, 8T] -> [128, 8T] (8 DMA loads)
        idx_s = const.tile([P, 8 * T], I16, name="idx_s")
        for r in range(8):
            nc.sync.dma_start(out=idx_s[16 * r:16 * (r + 1), :], in_=idx_in[:, :])

        def nodeblocks(step=P):
            for c0 in range(0, B, step):
                yield c0, min(step, B - c0)

        # ---- embedding: h0_T[96, B] = (h @ Wemb + bemb).T ----
        # computed directly transposed: lhsT=Wemb gives out [NHID, nodes].
        h_cur = hp.tile([NHID, B], F16, tag="h", name="h0")
        for c0, pn in nodeblocks(512):
            pe = pag.tile([NHID, 512], F32, tag="pa", name="pe")
            nc.tensor.matmul(pe[:, :pn], lhsT=Wemb_s[:], rhs=hT_s[:, c0:c0 + pn],
                             start=True, stop=True)
            nc.scalar.activation(out=h_cur[:, c0:c0 + pn], in_=pe[:, :pn],
                                 func=AF.Identity, bias=bemb_s[:, :1])

        # ---- layers ----
        for li in range(NL):
            gauss_s = gl_s[li]
            # xg block (node-major fp16 rows for the gather) + all-gather
            xg_src = dramp.tile([B, XGW], F16, tag="xgs", name=f"xg_src{li}")
            xg_full = dramp.tile([cfg.N, XGW], F16, tag="xgf", addr_space="Shared",
                                 name=f"xg_full{li}")
            for c0, pn in nodeblocks():
                px = pmm.tile([P, XGW], F32, tag="mm", name="px")
                nc.tensor.matmul(px[:pn, :], lhsT=h_cur[:, c0:c0 + pn],
                                 rhs=Wg_s[:, li * XGW:(li + 1) * XGW],
                                 start=True, stop=True)
                xs = sbp.tile([P, XGW], F16, tag="xs", name="xs")
                nc.scalar.copy(out=xs[:pn, :], in_=px[:pn, :])
                nc.sync.dma_start(out=xg_src[c0:c0 + pn, :], in_=xs[:pn, :])
            nc.gpsimd.collective_compute(
                "AllGather", ALU.bypass,
                replica_groups=[list(range(C))],
                ins=[xg_src[:, :]],
                outs=[xg_full[:, :]],
            )

            # edge aggregation, half-major: gather calls span dest blocks so
            # each call carries the full MAXT tiles (fewer SWDGE fixed costs).
            # Pass h=0 parks per-block partial sums in an SBUF accumulator;
            # pass h=1 adds the root term and finishes with the epilogue.
            h_new = hp.tile([NHID, B], F16, tag="h", name=f"h{li + 1}")
            aggP = hp.tile([NHID, B], F32, tag="aggp", name=f"agg{li}")
            # tile index -> (dest block, first/last tile of that block)
            blk_of = []
            for h in (0, 1):
                for nt in range(NBLK):
                    blk_of += [(h, nt)] * K2[h * NBLK + nt]
            for h in (0, 1):
                t0h = toff[h * NBLK]
                t1h = toff[(h + 1) * NBLK] if h == 0 else T
                # gather whole half in MAXT-tile calls (block-agnostic)
                xjs = {}
                for tcall in range(t0h, t1h, MAXT):
                    kc = min(MAXT, t1h - tcall)
                    xj = xjp.tile([P, MAXT * XGW], F16, tag="xj", name="xj")
                    out_ap = xj[:, 0:kc * XGW].rearrange("p (k e) -> p k e", e=XGW)
                    nc.gpsimd.dma_gather(
                        out_ap, xg_full[h * HALF:(h + 1) * HALF, :],
                        idx_s[:, tcall * 8:(tcall + kc) * 8],
                        kc * P, kc * P, XGW)
                    xjs[tcall] = xj
                pa = None
                for t in range(t0h, t1h):
                    _, nt = blk_of[t]
                    c0 = nt * P
                    pn = min(P, B - c0)
                    first = (t == t0h) or (blk_of[t - 1][1] != nt)
                    last = (t == t1h - 1) or (blk_of[t + 1][1] != nt)
                    if first:
                        pa = pag.tile([NHID, P], F32, tag="pa", name="pa")
                        if h == 1:
                            nc.tensor.matmul(
                                pa[:, :pn],
                                lhsT=Wr_s[:, li * NHID:(li + 1) * NHID],
                                rhs=h_cur[:, c0:c0 + pn], start=True, stop=False)
                    sel = selp.tile([P, P], F16, tag="sel", name="sel")
                    nc.vector.tensor_scalar(
                        out=sel[:], in0=R_s[:],
                        scalar1=dl_s[:, t:t + 1], scalar2=gauss_s[:, t:t + 1],
                        op0=ALU.is_equal, op1=ALU.mult)
                    xj = xjs[t0h + ((t - t0h) // MAXT) * MAXT]
                    k = (t - t0h) % MAXT
                    nc.tensor.matmul(pa[:, :pn], lhsT=xj[:, k * XGW:k * XGW + NHID],
                                     rhs=sel[:, :pn],
                                     start=(h == 0 and first), stop=last)
                    if last:
                        if h == 0:
                            nc.scalar.copy(out=aggP[:, c0:c0 + pn], in_=pa[:, :pn])
                        else:
                            sm = sbp.tile([NHID, P], F32, tag="sm", name="sm")
                            nc.vector.tensor_tensor(out=sm[:, :pn], in0=pa[:, :pn],
                                                    in1=aggP[:, c0:c0 + pn],
                                                    op=ALU.add)
                            rl = sbp.tile([NHID, P], F16, tag="rl", name="rl")
                            nc.scalar.activation(out=rl[:, :pn], in_=sm[:, :pn],
                                                 func=AF.Relu,
                                                 bias=bconv_s[:, li:li + 1])
                            nc.vector.tensor_tensor(out=h_new[:, c0:c0 + pn],
                                                    in0=rl[:, :pn],
                                                    in1=h_cur[:, c0:c0 + pn],
                                                    op=ALU.add)
                            if li == NL - 1:
                                # fused output head
                                po = pmm.tile([P, XGW], F32, tag="mm", name="po")
                                nc.tensor.matmul(po[:pn, :NCLASS],
                                                 lhsT=h_new[:, c0:c0 + pn],
                                                 rhs=Wo_s[:], start=True, stop=True)
                                ob = sbp.tile([P, NCLASS], F32, tag="ob", name="ob")
                                nc.vector.tensor_tensor(out=ob[:pn, :],
                                                        in0=po[:pn, :NCLASS],
                                                        in1=bout_s[:pn, :],
                                                        op=ALU.add)
                                nc.sync.dma_start(out=out_ext[c0:c0 + pn, :],
                                                  in_=ob[:pn, :])
            h_cur = h_new

    nc.finalize()
    return nc


def make_in_maps(cfg, prep, h, W_emb, b_emb, Wg, Wroot, b_conv, W_out, b_out):
    C, B, NL, NHID, XGW = cfg.C, cfg.B, cfg.NL, cfg.NHID, cfg.XGW
    h = np.asarray(h, np.float32)
    Wg_p = np.zeros((NL, NHID, XGW), np.float16)
    Wg_p[:, :, :NHID] = np.asarray(Wg, np.float32).reshape(NL, NHID, NHID)
    R = np.tile(np.arange(P, dtype=np.float32), (P, 1))
    common = dict(
        R=np.ascontiguousarray(R),
        Wemb=np.ascontiguousarray(np.asarray(W_emb, np.float16)),
        Wg=np.ascontiguousarray(Wg_p),
        Wr=np.ascontiguousarray(np.asarray(Wroot, np.float16)),
        Wo=np.ascontiguousarray(np.asarray(W_out, np.float16)),
        bemb=np.ascontiguousarray(np.asarray(b_emb, np.float32)[:, None]),
        bconv=np.ascontiguousarray(np.asarray(b_conv, np.float32).T),
        bout=np.ascontiguousarray(np.tile(np.asarray(b_out, np.float32), (P, 1))),
    )
    in_maps = []
    for m in range(C):
        d = dict(common)
        d["hT"] = np.ascontiguousarray(h[m * B:(m + 1) * B, :].T.astype(np.float16))
        d["idx16"] = np.ascontiguousarray(prep["idxA"][m])
        d["ed"] = np.ascontiguousarray(prep["edA"][m])
        in_maps.append(d)
    return in_maps


def run(cfg, inputs, trace=False):
    prep = host_prep(cfg, inputs["edge_index"], inputs["edge_weight"],
                     inputs["Wp"], inputs["bp"], inputs["mu"], inputs["sigma"])
    nc = build(cfg, prep)
    in_maps = make_in_maps(cfg, prep, inputs["h"], inputs["W_emb"], inputs["b_emb"],
                           inputs["Wg"], inputs["Wroot"], inputs["b_conv"],
                           inputs["W_out"], inputs["b_out"])
    res = bass_utils.run_bass_kernel_spmd(nc, in_maps, core_ids=list(range(cfg.C)),
                                          trace=trace)
    out = np.concatenate([res.results[m]["out"] for m in range(cfg.C)], axis=0)
    return out.astype(np.float32), res


def kernel(**inputs):
    cfg = Cfg()
    out, _ = run(cfg, inputs, trace=False)
    return out
